# revision 1
# baseline (speedup 1.0000x reference)
"""Multi-head attention (RoPE, causal) Trainium2 Bass kernel, 8-way sharded.

Sharding: core c handles batch b = c//2 and head-group hg = c%2 (8 of 16
heads). Each core computes Q/K/V projections for its head slice in
transposed layout (QT/KT: [hd, l] with de-interleaved RoPE pairs), runs
causal flash-style attention per head with scores kept transposed
(S^T[k, q], keys on partitions), and a partial output projection
out^T = Wo_slice @ attn^T. Host sums the two head-group partials per batch,
transposes back, and adds the output bias.

Matmul operands are bf16 (fp32 accumulation in PSUM); elementwise math
(RoPE, softmax scaling/normalization) stays fp32.
"""

from contextlib import ExitStack

import ml_dtypes
import numpy as np

import concourse.bass as bass
import concourse.mybir as mybir
import concourse.tile as tile
from concourse import bacc
from concourse.bass_utils import run_bass_kernel_spmd

F32 = mybir.dt.float32
BF16 = mybir.dt.bfloat16
AF = mybir.ActivationFunctionType
ALU = mybir.AluOpType

B, L, D = 4, 2048, 1024
H, HD = 16, 64          # global heads, head dim
HPC = 8                 # heads per core
DH = HPC * HD           # 512: per-core projected width
KT = L // 128           # 16 key tiles
NCORES = 8
ROPE_BASE = 10000.0

_cache: dict = {}


def _build(compile=True):
    if "nc" in _cache:
        return _cache["nc"]

    nc = bacc.Bacc("TRN2", target_bir_lowering=False, debug=False)

    qT = nc.dram_tensor("qT", [D, L], BF16, kind="ExternalInput").ap()
    kT = nc.dram_tensor("kT", [D, L], BF16, kind="ExternalInput").ap()
    vT = nc.dram_tensor("vT", [D, L], BF16, kind="ExternalInput").ap()
    wqT = nc.dram_tensor("wqT", [D, DH], BF16, kind="ExternalInput").ap()
    wkT = nc.dram_tensor("wkT", [D, DH], BF16, kind="ExternalInput").ap()
    wvT = nc.dram_tensor("wvT", [D, DH], BF16, kind="ExternalInput").ap()
    woT = nc.dram_tensor("woT", [DH, D], BF16, kind="ExternalInput").ap()
    bqc = nc.dram_tensor("bqc", [128, 4], F32, kind="ExternalInput").ap()
    bkc = nc.dram_tensor("bkc", [128, 4], F32, kind="ExternalInput").ap()
    bvc = nc.dram_tensor("bvc", [1, DH], F32, kind="ExternalInput").ap()
    cosP = nc.dram_tensor("cosP", [128, L], F32, kind="ExternalInput").ap()
    sinP = nc.dram_tensor("sinP", [128, L], F32, kind="ExternalInput").ap()
    maskc = nc.dram_tensor("maskc", [128, 128], BF16, kind="ExternalInput").ap()
    outT = nc.dram_tensor("outT", [D, L], F32, kind="ExternalOutput").ap()

    with tile.TileContext(nc) as tc, ExitStack() as ctx:
        const = ctx.enter_context(tc.tile_pool(name="const", bufs=1))

        mask_t = const.tile([128, 128], BF16, tag="mask")
        nc.sync.dma_start(mask_t[:], maskc[:])
        bq_t = const.tile([128, 4], F32, tag="bq")
        nc.sync.dma_start(bq_t[:], bqc[:])
        bk_t = const.tile([128, 4], F32, tag="bk")
        nc.sync.dma_start(bk_t[:], bkc[:])
        bv_sb = const.tile([1, DH], F32, tag="bv")
        nc.sync.dma_start(bv_sb[:], bvc[:])
        bv_b = const.tile([128, DH], F32, tag="bvb")
        nc.gpsimd.partition_broadcast(bv_b[:], bv_sb[:])

        with ExitStack() as bctx:
            pqk = bctx.enter_context(tc.tile_pool(name="pqk", bufs=1))
            pva = bctx.enter_context(tc.tile_pool(name="pva", bufs=1))
            pot = bctx.enter_context(tc.tile_pool(name="pot", bufs=1))
            qt_m = [pqk.tile([128, L], BF16, tag=f"qt{m}", name=f"qt{m}")
                    for m in range(4)]
            kt_m = [pqk.tile([128, L], BF16, tag=f"kt{m}", name=f"kt{m}")
                    for m in range(4)]
            ot_m = [pot.tile([128, L], BF16, tag=f"ot{m}", name=f"ot{m}")
                    for m in range(4)]
            va = [pva.tile([128, HPC * 65], BF16, tag=f"va{t}", name=f"va{t}")
                  for t in range(KT)]
            for t in range(KT):
                ones_view = va[t].rearrange("p (h x) -> p h x", x=65)[:, :, 64:65]
                nc.gpsimd.memset(ones_view, 1.0)

            # ---------------- Phase A1: Q/K projections + RoPE ------------
            with ExitStack() as actx:
                pin = actx.enter_context(tc.tile_pool(name="pin", bufs=2))
                pw = actx.enter_context(tc.tile_pool(name="pw", bufs=1))
                ptrig = actx.enter_context(tc.tile_pool(name="ptrig", bufs=1))
                praw = actx.enter_context(tc.tile_pool(name="praw", bufs=2))
                psw = actx.enter_context(tc.tile_pool(name="psw", bufs=2))
                psA = actx.enter_context(
                    tc.tile_pool(name="psA", bufs=2, space="PSUM"))

                cos_t = ptrig.tile([128, L], F32, tag="cos")
                nc.sync.dma_start(cos_t[:], cosP[:])
                sin_t = ptrig.tile([128, L], F32, tag="sin")
                nc.sync.dma_start(sin_t[:], sinP[:])

                for xT, wT, bias_t, dst in (
                    (kT, wkT, bk_t, kt_m),
                    (qT, wqT, bq_t, qt_m),
                ):
                    w_sb = []
                    for kk in range(8):
                        w = pw.tile([128, DH], BF16, tag=f"w{kk}")
                        nc.sync.dma_start(w[:], wT[kk * 128:(kk + 1) * 128, :])
                        w_sb.append(w)
                    # fp32 projection into traw per m-tile, then RoPE
                    traws = {}
                    for nj in range(4):
                        xp = []
                        for kk in range(8):
                            x = pin.tile([128, 512], BF16, tag=f"x{kk}",
                                         name=f"x{kk}")
                            nc.sync.dma_start(
                                x[:], xT[kk * 128:(kk + 1) * 128,
                                         nj * 512:(nj + 1) * 512])
                            xp.append(x)
                        for mi in range(4):
                            if nj == 0:
                                traws[mi] = praw.tile(
                                    [128, L], F32, tag=f"traw{mi}",
                                    name=f"traw{mi}")
                            ps = psA.tile([128, 512], F32, tag=f"pp{mi}",
                                          name=f"pp{mi}")
                            for kk in range(8):
                                nc.tensor.matmul(
                                    ps[:],
                                    w_sb[kk][:, mi * 128:(mi + 1) * 128],
                                    xp[kk][:],
                                    start=(kk == 0), stop=(kk == 7),
                                )
                            nc.vector.tensor_scalar_add(
                                traws[mi][:, nj * 512:(nj + 1) * 512],
                                ps[:], bias_t[:, mi:mi + 1])
                    for mi in range(4):
                        traw = traws[mi]
                        sw = psw.tile([128, L], F32, tag="sw", name="sw")
                        for blk in range(4):
                            srcb = blk ^ 1
                            nc.sync.dma_start(
                                sw[blk * 32:(blk + 1) * 32, :],
                                traw[srcb * 32:(srcb + 1) * 32, :])
                        nc.vector.tensor_mul(sw[:], sw[:], sin_t[:])
                        nc.vector.tensor_mul(traw[:], traw[:], cos_t[:])
                        nc.vector.tensor_add(dst[mi][:], traw[:], sw[:])

            # ---------------- Phase A2: V projection (natural layout) -----
            with ExitStack() as actx:
                pinv = actx.enter_context(tc.tile_pool(name="pinv", bufs=2))
                pwv = actx.enter_context(tc.tile_pool(name="pwv", bufs=1))
                psV = actx.enter_context(
                    tc.tile_pool(name="psV", bufs=2, space="PSUM"))

                wv_sb = []
                for kk in range(8):
                    w = pwv.tile([128, DH], BF16, tag=f"wv{kk}")
                    nc.sync.dma_start(w[:], wvT[kk * 128:(kk + 1) * 128, :])
                    wv_sb.append(w)
                for ltg in range(4):           # groups of 4 l-tiles
                    vp = []
                    for kk in range(8):
                        x = pinv.tile([128, 512], BF16, tag=f"vx{kk}",
                                      name=f"vx{kk}")
                        nc.sync.dma_start(
                            x[:], vT[kk * 128:(kk + 1) * 128,
                                     ltg * 512:(ltg + 1) * 512])
                        vp.append(x)
                    for li in range(4):
                        lt = ltg * 4 + li
                        ps = psV.tile([128, DH], F32, tag=f"pv{li}",
                                      name=f"pv{li}")
                        for kk in range(8):
                            nc.tensor.matmul(
                                ps[:],
                                vp[kk][:, li * 128:(li + 1) * 128],
                                wv_sb[kk][:],
                                start=(kk == 0), stop=(kk == 7),
                            )
                        out_view = va[lt].rearrange(
                            "p (h x) -> p h x", x=65)[:, :, 0:64]
                        nc.vector.tensor_add(out_view, ps[:], bv_b[:])

            # ---------------- Phase B: attention per head -----------------
            with ExitStack() as bctx2:
                psS = bctx2.enter_context(
                    tc.tile_pool(name="psS", bufs=2, space="PSUM"))
                psO = bctx2.enter_context(
                    tc.tile_pool(name="psO", bufs=1, space="PSUM"))
                ppp = bctx2.enter_context(tc.tile_pool(name="ppp", bufs=3))
                pnm = bctx2.enter_context(tc.tile_pool(name="pnm", bufs=2))
                pdr = bctx2.enter_context(
                    tc.tile_pool(name="pdr", bufs=2, space="DRAM"))

                for h in range(HPC):
                    mi, pb = h // 2, (h % 2) * 64
                    o_ps = psO.tile([128, L], F32, tag="O", name="o_ps")
                    for kt_i in range(KT):
                        qoff = kt_i * 128
                        w = L - qoff
                        # S^T chunks of <=1024 queries (2 psum banks,
                        # double-buffered): exp drains one chunk while the
                        # next is computed
                        nch = (w + 1023) // 1024
                        for c in range(nch):
                            cw = min(1024, w - c * 1024)
                            s_ps = psS.tile([128, 1024], F32, tag="S",
                                            name="s_ps")
                            for c2 in range((cw + 511) // 512):
                                c2w = min(512, cw - c2 * 512)
                                off = c * 1024 + c2 * 512
                                nc.tensor.matmul(
                                    s_ps[:, c2 * 512:c2 * 512 + c2w],
                                    kt_m[mi][pb:pb + 64, qoff:qoff + 128],
                                    qt_m[mi][pb:pb + 64,
                                             qoff + off:qoff + off + c2w],
                                    start=True, stop=True,
                                )
                            pt = ppp.tile([128, 1024], BF16, tag="P",
                                          name="pt")
                            nc.scalar.activation(pt[:, :cw], s_ps[:, :cw],
                                                 AF.Exp, scale=0.125)
                            if c == 0:
                                nc.vector.tensor_mul(
                                    pt[:, 0:128], pt[:, 0:128], mask_t[:])
                            # PV pieces covered by this chunk
                            glo = qoff + c * 1024
                            ghi = qoff + c * 1024 + cw
                            for qc in range(glo // 512, (ghi + 511) // 512):
                                lo = max(qc * 512, glo)
                                hi = min(qc * 512 + 512, ghi)
                                nc.tensor.matmul(
                                    o_ps[0:65, lo:hi],
                                    va[kt_i][:, h * 65:h * 65 + 65],
                                    pt[:, lo - glo:hi - glo],
                                    start=(kt_i == 0),
                                    stop=(kt_i == min(15, 4 * qc + 3)),
                                    skip_group_check=True,
                                )
                    # normalize: o * (1/colsum) -> bf16 ot slot.
                    # reciprocal via [128, 16] partition spread (DVE recip is
                    # per-lane serial); reshape through a DRAM bounce since
                    # SBUF DMAs cannot cross partitions in flat layout.
                    rs = pnm.tile([1, L], F32, tag="rs", name="rs")
                    nc.vector.tensor_copy(rs[:], o_ps[64:65, :])
                    b1 = pdr.tile([1, L], F32, tag="b1", name="b1")
                    nc.sync.dma_start(b1[:], rs[:])
                    r2 = pnm.tile([128, L // 128], F32, tag="r2", name="r2")
                    nc.sync.dma_start(
                        r2[:], b1.rearrange("o (p x) -> (o p) x", p=128))
                    nc.vector.reciprocal(r2[:], r2[:])
                    b2 = pdr.tile([1, L], F32, tag="b2", name="b2")
                    nc.sync.dma_start(
                        b2[:].rearrange("o (p x) -> (o p) x", p=128), r2[:])
                    rs2 = pnm.tile([1, L], F32, tag="rs2", name="rs2")
                    nc.sync.dma_start(rs2[:], b2[:])
                    rcb = pnm.tile([64, L], F32, tag="rcb", name="rcb")
                    nc.gpsimd.partition_broadcast(rcb[:], rs2[:], channels=64)
                    nc.vector.tensor_mul(
                        ot_m[mi][pb:pb + 64, :], o_ps[0:64, :], rcb[:])

            # ------------- Phase C: output projection ---------------------
            with ExitStack() as cctx:
                pwo = cctx.enter_context(tc.tile_pool(name="pwo", bufs=1))
                pout = cctx.enter_context(tc.tile_pool(name="pout", bufs=2))
                psC = cctx.enter_context(
                    tc.tile_pool(name="psC", bufs=2, space="PSUM"))

                wo_sb = []
                for kti in range(4):
                    w = pwo.tile([128, D], BF16, tag=f"wo{kti}")
                    nc.sync.dma_start(w[:], woT[kti * 128:(kti + 1) * 128, :])
                    wo_sb.append(w)
                for mo in range(8):
                    ps = psC.tile([128, L], F32, tag="C", name="psc")
                    for nj in range(4):
                        for kti in range(4):
                            nc.tensor.matmul(
                                ps[:, nj * 512:(nj + 1) * 512],
                                wo_sb[kti][:, mo * 128:(mo + 1) * 128],
                                ot_m[kti][:, nj * 512:(nj + 1) * 512],
                                start=(kti == 0), stop=(kti == 3),
                            )
                    osb = pout.tile([128, L], F32, tag="osb", name="osb")
                    nc.scalar.copy(osb[:], ps[:])
                    nc.sync.dma_start(outT[mo * 128:(mo + 1) * 128, :],
                                      osb[:])

    if compile:
        nc.compile()
        _cache["nc"] = nc
    return nc


def _prep(q, k, v, Wq, bq, Wk, bk, Wv, bv, Wo, bo):
    """Build the 8 per-core input maps (host-side shard + layout prep)."""
    bf16 = ml_dtypes.bfloat16
    # de-interleave permutation within each head: evens then odds
    perm = np.concatenate([np.arange(0, HD, 2), np.arange(1, HD, 2)])

    # RoPE tables in de-interleaved layout, tiled x2 over partitions
    inv_freq = 1.0 / (ROPE_BASE ** (np.arange(0, HD // 2, dtype=np.float64)
                                    * 2.0 / HD))
    t = np.arange(L, dtype=np.float64)
    freqs = inv_freq[:, None] * t[None, :]            # [32, L]
    cos64 = np.cos(np.concatenate([freqs, freqs], axis=0))   # [64, L]
    sin64 = np.sin(np.concatenate([freqs, freqs], axis=0))
    sin64[:32] *= -1.0
    cosP = np.tile(cos64, (2, 1)).astype(np.float32)  # [128, L]
    sinP = np.tile(sin64, (2, 1)).astype(np.float32)

    # causal mask in S^T space: keep k <= q
    kk, qq = np.meshgrid(np.arange(128), np.arange(128), indexing="ij")
    mask = (kk <= qq).astype(bf16)

    qTb = [np.ascontiguousarray(q[b_i].T.astype(bf16)) for b_i in range(B)]
    kTb = [np.ascontiguousarray(k[b_i].T.astype(bf16)) for b_i in range(B)]
    vTb = [np.ascontiguousarray(v[b_i].T.astype(bf16)) for b_i in range(B)]

    in_maps = []
    for c in range(NCORES):
        b_i, hg = c // 2, c % 2
        rows = hg * DH + (np.arange(DH).reshape(HPC, HD)[:, perm]).reshape(-1)
        in_maps.append({
            "qT": qTb[b_i],
            "kT": kTb[b_i],
            "vT": vTb[b_i],
            "wqT": np.ascontiguousarray(Wq[rows, :].T.astype(bf16)),
            "wkT": np.ascontiguousarray(Wk[rows, :].T.astype(bf16)),
            "wvT": np.ascontiguousarray(
                Wv[hg * DH:(hg + 1) * DH, :].T.astype(bf16)),
            "woT": np.ascontiguousarray(
                Wo[:, hg * DH:(hg + 1) * DH].T.astype(bf16)),
            "bqc": np.ascontiguousarray(bq[rows].reshape(4, 128).T),
            "bkc": np.ascontiguousarray(bk[rows].reshape(4, 128).T),
            "bvc": np.ascontiguousarray(
                bv[hg * DH:(hg + 1) * DH].reshape(1, DH)),
            "cosP": cosP,
            "sinP": sinP,
            "maskc": mask,
        })
    return in_maps


def _assemble(results, bo):
    out = np.empty((B, L, D), dtype=np.float32)
    for b_i in range(B):
        acc = results[2 * b_i]["outT"] + results[2 * b_i + 1]["outT"]
        out[b_i] = acc.T + bo[None, :]
    return out


def kernel(q, k, v, Wq, bq, Wk, bk, Wv, bv, Wo, bo):
    q = np.asarray(q, dtype=np.float32)
    k = np.asarray(k, dtype=np.float32)
    v = np.asarray(v, dtype=np.float32)
    Wq = np.asarray(Wq, dtype=np.float32)
    Wk = np.asarray(Wk, dtype=np.float32)
    Wv = np.asarray(Wv, dtype=np.float32)
    Wo = np.asarray(Wo, dtype=np.float32)
    bq = np.asarray(bq, dtype=np.float32)
    bk = np.asarray(bk, dtype=np.float32)
    bv = np.asarray(bv, dtype=np.float32)
    bo = np.asarray(bo, dtype=np.float32)

    nc = _build()
    in_maps = _prep(q, k, v, Wq, bq, Wk, bk, Wv, bv, Wo, bo)
    res = run_bass_kernel_spmd(nc, in_maps, core_ids=list(range(NCORES)))
    return _assemble(res.results, bo)



# revision 3
# speedup vs baseline: 1.3556x; 1.3556x over previous
"""Multi-head attention (RoPE, causal) Trainium2 Bass kernel, 8-way sharded.

Sharding: core c handles batch b = c//2 and head-group hg = c%2 (8 of 16
heads). Each core computes Q/K/V projections for its head slice in
transposed layout (QT/KT: [hd, l]), runs causal flash-style attention per
head with scores kept transposed (S^T[k, q], keys on partitions), and a
partial output projection out^T = Wo_slice @ attn^T. Host sums the two
head-group partials per batch, transposes back, and adds the output bias.

Key layout tricks:
- RoPE pairs are de-interleaved at 16-row granularity so the rotate-half
  partner swap is an intra-quadrant DVE stream_shuffle (no DMA bounce).
- V tiles carry 64 replicated ones-columns per head, so the PV matmul
  emits the softmax denominator already broadcast across 64 partitions;
  normalization is reciprocal_approx_fast + one tensor multiply.
- Attention is processed per (head, 1024-query vhead) with double-buffered
  PSUM score/output tiles and software-pipelined S -> exp -> PV emission.
"""

from contextlib import ExitStack

import ml_dtypes
import numpy as np

import concourse.bass as bass
import concourse.mybir as mybir
import concourse.tile as tile
from concourse import bacc
from concourse.bass_utils import run_bass_kernel_spmd

F32 = mybir.dt.float32
BF16 = mybir.dt.bfloat16
AF = mybir.ActivationFunctionType
ALU = mybir.AluOpType

B, L, D = 4, 2048, 1024
H, HD = 16, 64          # global heads, head dim
HPC = 8                 # heads per core
DH = HPC * HD           # 512: per-core projected width
KT = L // 128           # 16 key tiles
VH = 1024               # vhead width (queries per psum O tile)
NCORES = 8
ROPE_BASE = 10000.0

# rotate-half partner swap within each 32-partition quadrant
SHUF_MASK = [(i + 16) % 32 for i in range(32)]

_cache: dict = {}


def _build(compile=True):
    if "nc" in _cache:
        return _cache["nc"]

    nc = bacc.Bacc("TRN2", target_bir_lowering=False, debug=False)

    qT = nc.dram_tensor("qT", [D, L], BF16, kind="ExternalInput").ap()
    kT = nc.dram_tensor("kT", [D, L], BF16, kind="ExternalInput").ap()
    vT = nc.dram_tensor("vT", [D, L], BF16, kind="ExternalInput").ap()
    wqT = nc.dram_tensor("wqT", [D, DH], BF16, kind="ExternalInput").ap()
    wkT = nc.dram_tensor("wkT", [D, DH], BF16, kind="ExternalInput").ap()
    wvT = nc.dram_tensor("wvT", [D, DH], BF16, kind="ExternalInput").ap()
    woT = nc.dram_tensor("woT", [DH, D], BF16, kind="ExternalInput").ap()
    bqc = nc.dram_tensor("bqc", [128, 4], F32, kind="ExternalInput").ap()
    bkc = nc.dram_tensor("bkc", [128, 4], F32, kind="ExternalInput").ap()
    bvc = nc.dram_tensor("bvc", [1, DH], F32, kind="ExternalInput").ap()
    cosP = nc.dram_tensor("cosP", [128, L], BF16, kind="ExternalInput").ap()
    sinP = nc.dram_tensor("sinP", [128, L], BF16, kind="ExternalInput").ap()
    maskc = nc.dram_tensor("maskc", [128, 128], BF16, kind="ExternalInput").ap()
    outT = nc.dram_tensor("outT", [D, L], F32, kind="ExternalOutput").ap()

    with tile.TileContext(nc) as tc, ExitStack() as ctx:
        const = ctx.enter_context(tc.tile_pool(name="const", bufs=1))

        mask_t = const.tile([128, 128], BF16, tag="mask")
        nc.sync.dma_start(mask_t[:], maskc[:])
        bq_t = const.tile([128, 4], F32, tag="bq")
        nc.sync.dma_start(bq_t[:], bqc[:])
        bk_t = const.tile([128, 4], F32, tag="bk")
        nc.sync.dma_start(bk_t[:], bkc[:])
        bv_sb = const.tile([1, DH], F32, tag="bv")
        nc.sync.dma_start(bv_sb[:], bvc[:])
        bv_b = const.tile([128, DH], F32, tag="bvb")
        nc.gpsimd.partition_broadcast(bv_b[:], bv_sb[:])
        cos_t = const.tile([128, L], BF16, tag="cos")
        nc.sync.dma_start(cos_t[:], cosP[:])
        sin_t = const.tile([128, L], BF16, tag="sin")
        nc.sync.dma_start(sin_t[:], sinP[:])

        with ExitStack() as bctx:
            pqk = bctx.enter_context(tc.tile_pool(name="pqk", bufs=1))
            pva = bctx.enter_context(tc.tile_pool(name="pva", bufs=1))
            pot = bctx.enter_context(tc.tile_pool(name="pot", bufs=1))
            qt_m = [pqk.tile([128, L], BF16, tag=f"qt{m}", name=f"qt{m}")
                    for m in range(4)]
            kt_m = [pqk.tile([128, L], BF16, tag=f"kt{m}", name=f"kt{m}")
                    for m in range(4)]
            ot_m = [pot.tile([128, L], BF16, tag=f"ot{m}", name=f"ot{m}")
                    for m in range(4)]
            # va[kt]: per head h, cols [h*128, h*128+64) = projected V dims,
            # cols [h*128+64, h*128+128) = ones (softmax denominator rows)
            va = [pva.tile([128, HPC * 128], BF16, tag=f"va{t}", name=f"va{t}")
                  for t in range(KT)]
            for t in range(KT):
                ones_view = va[t].rearrange("p (h x) -> p h x", x=128)[:, :, 64:128]
                nc.gpsimd.memset(ones_view, 1.0)

            # ---------------- Phase A1: Q/K projections + RoPE ------------
            with ExitStack() as actx:
                pin = actx.enter_context(tc.tile_pool(name="pin", bufs=2))
                pw = actx.enter_context(tc.tile_pool(name="pw", bufs=2))
                praw = actx.enter_context(tc.tile_pool(name="praw", bufs=1))
                psw = actx.enter_context(tc.tile_pool(name="psw", bufs=2))
                psA = actx.enter_context(
                    tc.tile_pool(name="psA", bufs=2, space="PSUM"))

                for xT, wT, bias_t, dst in (
                    (kT, wkT, bk_t, kt_m),
                    (qT, wqT, bq_t, qt_m),
                ):
                    w_sb = []
                    for kk in range(8):
                        w = pw.tile([128, DH], BF16, tag=f"w{kk}")
                        nc.sync.dma_start(w[:], wT[kk * 128:(kk + 1) * 128, :])
                        w_sb.append(w)
                    # bf16 projection into traw per m-tile, then RoPE
                    traws = {}
                    for nj in range(4):
                        xp = []
                        for kk in range(8):
                            x = pin.tile([128, 512], BF16, tag=f"x{kk}",
                                         name=f"x{kk}")
                            nc.sync.dma_start(
                                x[:], xT[kk * 128:(kk + 1) * 128,
                                         nj * 512:(nj + 1) * 512])
                            xp.append(x)
                        for mi in range(4):
                            if nj == 0:
                                traws[mi] = praw.tile(
                                    [128, L], BF16, tag=f"traw{mi}",
                                    name=f"traw{mi}")
                            ps = psA.tile([128, 512], F32, tag=f"pp{mi}",
                                          name=f"pp{mi}")
                            for kk in range(8):
                                nc.tensor.matmul(
                                    ps[:],
                                    w_sb[kk][:, mi * 128:(mi + 1) * 128],
                                    xp[kk][:],
                                    start=(kk == 0), stop=(kk == 7),
                                )
                            nc.vector.tensor_scalar_add(
                                traws[mi][:, nj * 512:(nj + 1) * 512],
                                ps[:], bias_t[:, mi:mi + 1])
                    for mi in range(4):
                        traw = traws[mi]
                        sw = psw.tile([128, L], BF16, tag="sw", name="sw")
                        ss = psw.tile([128, L], BF16, tag="ss", name="ss")
                        nc.vector.stream_shuffle(sw[:], traw[:], SHUF_MASK)
                        nc.vector.tensor_mul(ss[:], sw[:], sin_t[:])
                        nc.vector.tensor_mul(sw[:], traw[:], cos_t[:])
                        nc.vector.tensor_add(dst[mi][:], sw[:], ss[:])

            # ---------------- Phase A2: V projection (natural layout) -----
            with ExitStack() as actx:
                pinv = actx.enter_context(tc.tile_pool(name="pinv", bufs=2))
                pwv = actx.enter_context(tc.tile_pool(name="pwv", bufs=1))
                psV = actx.enter_context(
                    tc.tile_pool(name="psV", bufs=2, space="PSUM"))

                wv_sb = []
                for kk in range(8):
                    w = pwv.tile([128, DH], BF16, tag=f"wv{kk}")
                    nc.sync.dma_start(w[:], wvT[kk * 128:(kk + 1) * 128, :])
                    wv_sb.append(w)
                for ltg in range(4):           # groups of 4 l-tiles
                    vp = []
                    for kk in range(8):
                        x = pinv.tile([128, 512], BF16, tag=f"vx{kk}",
                                      name=f"vx{kk}")
                        nc.sync.dma_start(
                            x[:], vT[kk * 128:(kk + 1) * 128,
                                     ltg * 512:(ltg + 1) * 512])
                        vp.append(x)
                    for li in range(4):
                        lt = ltg * 4 + li
                        ps = psV.tile([128, DH], F32, tag=f"pv{li}",
                                      name=f"pv{li}")
                        for kk in range(8):
                            nc.tensor.matmul(
                                ps[:],
                                vp[kk][:, li * 128:(li + 1) * 128],
                                wv_sb[kk][:],
                                start=(kk == 0), stop=(kk == 7),
                            )
                        out_view = va[lt].rearrange(
                            "p (h x) -> p h x", x=128)[:, :, 0:64]
                        nc.vector.tensor_add(out_view, ps[:], bv_b[:])

            # ---------------- Phase B: attention per (head, vhead) --------
            with ExitStack() as bctx2:
                psS = bctx2.enter_context(
                    tc.tile_pool(name="psS", bufs=2, space="PSUM"))
                psO = bctx2.enter_context(
                    tc.tile_pool(name="psO", bufs=2, space="PSUM"))
                ppp = bctx2.enter_context(tc.tile_pool(name="ppp", bufs=4))
                pnm = bctx2.enter_context(tc.tile_pool(name="pnm", bufs=2))

                for h in range(HPC):
                    mi, pb = h // 2, (h % 2) * 64
                    for vstart in (0, VH):
                        vend = vstart + VH
                        # chunk list: one psS tile per key tile
                        chunks = []
                        for kt_i in range(vend // 128):
                            qoff = kt_i * 128
                            qlo = max(qoff, vstart)
                            cw = vend - qlo
                            chunks.append((kt_i, qlo, cw))
                        n = len(chunks)
                        o_ps = psO.tile([128, VH], F32, tag="O", name="o_ps")
                        s_tiles = [None] * n
                        p_tiles = [None] * n

                        def emit_S(i):
                            kt_i, qlo, cw = chunks[i]
                            qoff = kt_i * 128
                            s_ps = psS.tile([128, VH], F32, tag="S",
                                            name="s_ps")
                            s_tiles[i] = s_ps
                            for lo in range(0, cw, 512):
                                w = min(512, cw - lo)
                                nc.tensor.matmul(
                                    s_ps[:, lo:lo + w],
                                    kt_m[mi][pb:pb + 64, qoff:qoff + 128],
                                    qt_m[mi][pb:pb + 64, qlo + lo:qlo + lo + w],
                                    start=True, stop=True,
                                )

                        def emit_exp(i):
                            kt_i, qlo, cw = chunks[i]
                            qoff = kt_i * 128
                            pt = ppp.tile([128, VH], BF16, tag="P", name="pt")
                            p_tiles[i] = pt
                            nc.scalar.activation(pt[:, :cw], s_tiles[i][:, :cw],
                                                 AF.Exp, scale=0.125)
                            if qlo == qoff:  # diagonal block: causal mask
                                nc.vector.tensor_mul(
                                    pt[:, 0:128], pt[:, 0:128], mask_t[:])

                        def emit_PV(i):
                            kt_i, qlo, cw = chunks[i]
                            pt = p_tiles[i]
                            lo_l = qlo - vstart          # local col offset
                            for qc in range(lo_l // 512,
                                            (lo_l + cw + 511) // 512):
                                lo = max(qc * 512, lo_l)
                                hi = min(qc * 512 + 512, lo_l + cw)
                                last_kt = min(KT - 1,
                                              (vstart + qc * 512 + 511) // 128)
                                nc.tensor.matmul(
                                    o_ps[:, lo:hi],
                                    va[kt_i][:, h * 128:(h + 1) * 128],
                                    pt[:, lo - lo_l:hi - lo_l],
                                    start=(kt_i == 0),
                                    stop=(kt_i == last_kt),
                                    skip_group_check=True,
                                )

                        emit_S(0)
                        for i in range(n):
                            emit_exp(i)
                            if i + 1 < n:
                                emit_S(i + 1)
                            emit_PV(i)

                        # normalize: denominator rows 64:128 (replicated by
                        # the ones-columns), reciprocal + multiply.
                        # reciprocal_approx_fast misreads PSUM at partition
                        # offset 64 on HW, so bounce through SBUF first.
                        dsb = pnm.tile([64, VH], F32, tag="dsb", name="dsb")
                        nc.vector.tensor_copy(dsb[:], o_ps[64:128, :])
                        rec = pnm.tile([64, VH], F32, tag="rec", name="rec")
                        nc.vector.reciprocal_approx_fast(rec[:], dsb[:])
                        nc.vector.tensor_mul(
                            ot_m[mi][pb:pb + 64, vstart:vend],
                            o_ps[0:64, :], rec[:])

            # ------------- Phase C: output projection ---------------------
            with ExitStack() as cctx:
                pwo = cctx.enter_context(tc.tile_pool(name="pwo", bufs=1))
                pout = cctx.enter_context(tc.tile_pool(name="pout", bufs=2))
                psC = cctx.enter_context(
                    tc.tile_pool(name="psC", bufs=2, space="PSUM"))

                wo_sb = []
                for kti in range(4):
                    w = pwo.tile([128, D], BF16, tag=f"wo{kti}")
                    nc.sync.dma_start(w[:], woT[kti * 128:(kti + 1) * 128, :])
                    wo_sb.append(w)
                for mo in range(8):
                    ps = psC.tile([128, L], F32, tag="C", name="psc")
                    for nj in range(4):
                        for kti in range(4):
                            nc.tensor.matmul(
                                ps[:, nj * 512:(nj + 1) * 512],
                                wo_sb[kti][:, mo * 128:(mo + 1) * 128],
                                ot_m[kti][:, nj * 512:(nj + 1) * 512],
                                start=(kti == 0), stop=(kti == 3),
                            )
                    osb = pout.tile([128, L], F32, tag="osb", name="osb")
                    nc.scalar.copy(osb[:], ps[:])
                    nc.sync.dma_start(outT[mo * 128:(mo + 1) * 128, :],
                                      osb[:])

    if compile:
        nc.compile()
        _cache["nc"] = nc
    return nc


def _prep(q, k, v, Wq, bq, Wk, bk, Wv, bv, Wo, bo):
    """Build the 8 per-core input maps (host-side shard + layout prep)."""
    bf16 = ml_dtypes.bfloat16
    # de-interleave at 16-pair granularity: rows r of a 64-row head hold
    #   [0:16)  even dims of pairs 0-15, [16:32) odd dims of pairs 0-15,
    #   [32:48) even dims of pairs 16-31, [48:64) odd dims of pairs 16-31
    # so the rotate-half partner lives 16 partitions away within the same
    # 32-partition quadrant (DVE stream_shuffle reachable).
    perm = np.concatenate([
        np.arange(0, 32, 2), np.arange(1, 32, 2),
        np.arange(32, 64, 2), np.arange(33, 64, 2),
    ])

    # RoPE tables matching that layout
    inv_freq = 1.0 / (ROPE_BASE ** (np.arange(0, HD // 2, dtype=np.float64)
                                    * 2.0 / HD))
    t = np.arange(L, dtype=np.float64)
    rows_p = np.empty(HD, dtype=np.int64)    # pair (freq) index per row
    sign = np.empty(HD, dtype=np.float64)    # sin sign per row
    for r in range(HD):
        qd, rr = r // 32, r % 32
        rows_p[r] = 16 * qd + (rr % 16)
        sign[r] = -1.0 if rr < 16 else 1.0
    freqs = inv_freq[rows_p][:, None] * t[None, :]      # [64, L]
    cos64 = np.cos(freqs)
    sin64 = np.sin(freqs) * sign[:, None]
    cosP = np.tile(cos64, (2, 1)).astype(bf16)          # [128, L]
    sinP = np.tile(sin64, (2, 1)).astype(bf16)

    # causal mask in S^T space: keep k <= q
    kk, qq = np.meshgrid(np.arange(128), np.arange(128), indexing="ij")
    mask = (kk <= qq).astype(bf16)

    qTb = [np.ascontiguousarray(q[b_i].T.astype(bf16)) for b_i in range(B)]
    kTb = [np.ascontiguousarray(k[b_i].T.astype(bf16)) for b_i in range(B)]
    vTb = [np.ascontiguousarray(v[b_i].T.astype(bf16)) for b_i in range(B)]

    in_maps = []
    for c in range(NCORES):
        b_i, hg = c // 2, c % 2
        rows = hg * DH + (np.arange(DH).reshape(HPC, HD)[:, perm]).reshape(-1)
        in_maps.append({
            "qT": qTb[b_i],
            "kT": kTb[b_i],
            "vT": vTb[b_i],
            "wqT": np.ascontiguousarray(Wq[rows, :].T.astype(bf16)),
            "wkT": np.ascontiguousarray(Wk[rows, :].T.astype(bf16)),
            "wvT": np.ascontiguousarray(
                Wv[hg * DH:(hg + 1) * DH, :].T.astype(bf16)),
            "woT": np.ascontiguousarray(
                Wo[:, hg * DH:(hg + 1) * DH].T.astype(bf16)),
            "bqc": np.ascontiguousarray(bq[rows].reshape(4, 128).T),
            "bkc": np.ascontiguousarray(bk[rows].reshape(4, 128).T),
            "bvc": np.ascontiguousarray(
                bv[hg * DH:(hg + 1) * DH].reshape(1, DH)),
            "cosP": cosP,
            "sinP": sinP,
            "maskc": mask,
        })
    return in_maps


def _assemble(results, bo):
    out = np.empty((B, L, D), dtype=np.float32)
    for b_i in range(B):
        acc = results[2 * b_i]["outT"] + results[2 * b_i + 1]["outT"]
        out[b_i] = acc.T + bo[None, :]
    return out


def kernel(q, k, v, Wq, bq, Wk, bk, Wv, bv, Wo, bo):
    q = np.asarray(q, dtype=np.float32)
    k = np.asarray(k, dtype=np.float32)
    v = np.asarray(v, dtype=np.float32)
    Wq = np.asarray(Wq, dtype=np.float32)
    Wk = np.asarray(Wk, dtype=np.float32)
    Wv = np.asarray(Wv, dtype=np.float32)
    Wo = np.asarray(Wo, dtype=np.float32)
    bq = np.asarray(bq, dtype=np.float32)
    bk = np.asarray(bk, dtype=np.float32)
    bv = np.asarray(bv, dtype=np.float32)
    bo = np.asarray(bo, dtype=np.float32)

    nc = _build()
    in_maps = _prep(q, k, v, Wq, bq, Wk, bk, Wv, bv, Wo, bo)
    res = run_bass_kernel_spmd(nc, in_maps, core_ids=list(range(NCORES)))
    return _assemble(res.results, bo)
